# revision 16
# baseline (speedup 1.0000x reference)
"""Sharded top-1 KNN (retrieval) on 8 TRN2 NeuronCores via Bass/Tile.

v3 strategy (hardcoded for x[2048,24,16], X_train[65536,384], Y_train[65536,24,1]):
  - Shard X_train rows across 8 cores (8192 rows each), rows permuted so that
    each pooled output column covers tt-adjacent rows (tt = ||t||^2).
  - All scoring in fp8 e4m3 (ml_dtypes.float8_e4m3 == TRN FP8_EXP4): the
    384-dim contraction runs as one DoubleRow matmul (dims 0..255, 2 fp8
    packed per PE cell -> 2x throughput) plus one plain fp8 matmul
    (dims 256..383), accumulated in PSUM fp32.  Host-side recall check:
    fp8 quantization noise (std ~0.5) vs top-1->top-8 pooled margin (~11)
    leaves ~0 miss probability; exact distances are recomputed on host for
    the surviving candidates, so the final output is exact.
  - PSUM is drained with a Scalar/Vector split: ScalarE ACTIVATE-copies even
    fills psum->sbuf bf16, VectorE tensor_tensor-max folds odd fills onto
    them (the only engines that can read PSUM).  This emits fold-2 pooled
    score columns; top-8 selection happens on HOST (saves MAX8/FIND_INDEX8
    ~39us of VectorE time), after DMA-ing the pooled bf16 scores out.
  - Host: rank pooled columns by P - mean_tt(group)/2, keep top-8 per core,
    expand 2 rows per column -> 8*8*2 = 128 candidates/query, recompute
    exact float64 distances, argmin (ties: smallest global index, matching
    jnp.argmin), return Y_train[best].
"""

import os
import sys

import numpy as np

for _p in ("/opt/trn_rl_repo",):
    if os.path.isdir(_p) and _p not in sys.path:
        sys.path.insert(0, _p)

import ml_dtypes  # noqa: E402

B, T, F = 2048, 24, 16
D = T * F  # 384
N = 65536
NCORES = 8
NS = N // NCORES  # 8192 rows per core
MT = B // 128  # 16 query tiles
NCHUNK = 512
NT = NS // NCHUNK  # 16 train chunks per core
NFILL = NT // 2  # 8 psum fills per m-tile, [128,1024] each
FOLD = 2  # rows pooled per output column (device-side)
NG = NS // FOLD  # 4096 pooled columns per core (per query)
TOPK = 8

_F8 = ml_dtypes.float8_e4m3
_BF16 = ml_dtypes.bfloat16


def build_nc(b=B, ns=NS):
    """Per-core Bass program (SPMD: same program, per-core inputs)."""
    import concourse.tile as tile
    from concourse import bacc, mybir

    mt = b // 128
    nt = ns // NCHUNK
    nfill = nt // 2

    nc = bacc.Bacc(None, target_bir_lowering=False)
    dt = mybir.dt
    # xT[ki, ko, b] = x[b, ko*128+ki]
    xT = nc.dram_tensor("xT", [128, 3, b], dt.float8e4, kind="ExternalInput")
    # XT[ki, ko, n] = Xs_dev[n, ko*128+ki]
    XT = nc.dram_tensor("XT", [128, 3, ns], dt.float8e4, kind="ExternalInput")
    # scores[q, v, j]: fill-pair v of query q's m-tile; fold-2 pooled column j
    OUT = nc.dram_tensor("scores", [b, nfill // 2, 1024], dt.float8e4,
                         kind="ExternalOutput")

    DR = mybir.MatmulPerfMode.DoubleRow
    MAX = mybir.AluOpType.max

    with tile.TileContext(nc) as tc:
        with (
            tc.tile_pool(name="wpool", bufs=1) as wpool,
            tc.tile_pool(name="ppool", bufs=1, space="PSUM") as ppool,
            tc.tile_pool(name="spool", bufs=3) as spool,
            tc.tile_pool(name="vpool", bufs=2) as vpool,
        ):
            # psum: 4 persistent tiles, rotated manually
            pts = [
                ppool.tile([128, 1024], dt.float32, name=f"pt{i}", tag=f"pt{i}")
                for i in range(4)
            ]
            # PE warmup: dummy matmuls during the input-DMA wait keep the HAM
            # activity monitor busy so real matmuls start at 2.4 GHz.
            dum = wpool.tile([128, 128], dt.float8e4, name="dum", tag="dum")
            nc.vector.memset(dum[:], 0.0)
            for i in range(40):
                nc.tensor.matmul(pts[0][:, 0:128], dum[:], dum[:],
                                 start=True, stop=True)

            xT_s = wpool.tile([128, 3, b], dt.float8e4, name="xT_s", tag="xT")
            nc.sync.dma_start(xT_s[:], xT[:])
            XT_s = wpool.tile([128, 3, ns], dt.float8e4, name="XT_s", tag="XT")
            # fill-granular input DMA so m=0 matmuls start early
            for f0 in range(nfill):
                csl = slice(f0 * 2 * NCHUNK, (f0 + 1) * 2 * NCHUNK)
                nc.sync.dma_start(XT_s[:, :, csl], XT[:, :, csl])

            for m in range(mt):
                msl = slice(m * 128, (m + 1) * 128)
                S = None
                V4 = vpool.tile([128, nfill // 2, 1024], dt.float8e4,
                                name="V4", tag="V4")
                for f in range(nfill):
                    pt = pts[f % 4]
                    # k-outer: both DR matmuls back-to-back, then both plain
                    # (same-mode neighbors pipeline better on the PE)
                    for j in range(2):
                        c = 2 * f + j
                        csl = slice(c * NCHUNK, (c + 1) * NCHUNK)
                        nc.tensor.matmul(
                            pt[:, j * NCHUNK : (j + 1) * NCHUNK],
                            xT_s[:, 0:2, msl], XT_s[:, 0:2, csl],
                            start=True, stop=False, perf_mode=DR,
                        )
                    for j in range(2):
                        c = 2 * f + j
                        csl = slice(c * NCHUNK, (c + 1) * NCHUNK)
                        nc.tensor.matmul(
                            pt[:, j * NCHUNK : (j + 1) * NCHUNK],
                            xT_s[:, 2:3, msl], XT_s[:, 2:3, csl],
                            start=False, stop=True,
                        )
                    if f % 2 == 0:
                        S = spool.tile([128, 1024], dt.float8e4, name="S", tag="S")
                        nc.scalar.copy(S[:], pt[:])
                    else:
                        nc.vector.tensor_tensor(
                            V4[:, f // 2, :], pt[:], S[:], op=MAX
                        )
                if not os.environ.get("KNN_NO_OUT_DMA"):
                    nc.sync.dma_start(OUT[msl, :, :], V4[:])
    nc.finalize()
    return nc


_NC = None


def _get_nc():
    global _NC
    if _NC is None:
        _NC = build_nc()
    return _NC


def _group_rows(ng=NG):
    """Device rows covered by pooled column G (before the tt permutation).

    Pool col of output[v, q, j]: G = v*1024 + j; rows are chunks
    {4v+p, 4v+2+p} (p = j//512) at col j%512.
    """
    G = np.arange(ng)
    v, j = G // 1024, G % 1024
    p, col = j // 512, j % 512
    c0 = 4 * v + p
    c1 = 4 * v + 2 + p
    return np.stack([c0 * NCHUNK + col, c1 * NCHUNK + col], axis=1)  # [NG, 2]


def _rank_to_dev(ns=NS):
    """Device row for each tt-sorted rank r: group G=r//2 gets ranks 2G,2G+1."""
    r = np.arange(ns)
    G, i = r // FOLD, r % FOLD
    v, j = G // 1024, G % 1024
    p, col = j // 512, j % 512
    c = 4 * v + 2 * i + p
    return c * NCHUNK + col


_R2D = _rank_to_dev()
_GROWS = _group_rows()


def _prep_core(Xs):
    """Per-core device layout + host-side metadata."""
    Xq = Xs.astype(_F8)
    ttq = (Xq.astype(np.float64) ** 2).sum(axis=1)
    order = np.argsort(ttq, kind="stable")  # rank -> original shard row
    perm = np.empty(NS, dtype=np.int64)  # device row -> original shard row
    perm[_R2D] = order
    Xdev = Xq[perm]  # [NS, D] fp8
    XT_dev = np.ascontiguousarray(
        Xdev.T.reshape(3, 128, NS).transpose(1, 0, 2)
    )  # [128, 3, NS]
    tt_dev = ttq[perm]
    ttg = tt_dev[_GROWS].mean(axis=1)  # [NG] mean tt per pooled column
    grows = perm[_GROWS]  # [NG, 2] original shard rows per pooled column
    return XT_dev, ttg.astype(np.float32), grows


def _refine(xf, X_train, Y_train, cand):
    """cand: [B, C] global candidate row indices (may repeat)."""
    b = cand.shape[0]
    cand = np.sort(cand, axis=1)
    best = np.empty(b, dtype=np.int64)
    xd = xf.astype(np.float64)
    step = 256
    for s in range(0, b, step):
        e = min(s + step, b)
        Xc = X_train[cand[s:e]].astype(np.float64)  # [q, C, D]
        diff = xd[s:e, None, :] - Xc
        d2 = np.einsum("qcd,qcd->qc", diff, diff)
        best[s:e] = cand[s:e][np.arange(e - s), np.argmin(d2, axis=1)]
    return Y_train[best].astype(np.float32)


def kernel(x, X_train, Y_train, _trace=False, _tmpdir=None):
    from concourse.bass_utils import run_bass_kernel_spmd

    x = np.asarray(x, dtype=np.float32)
    X_train = np.asarray(X_train, dtype=np.float32)
    Y_train = np.asarray(Y_train, dtype=np.float32)
    xf = x.reshape(B, D)

    xq = xf.astype(_F8)
    xT_kio = np.ascontiguousarray(xq.T.reshape(3, 128, B).transpose(1, 0, 2))

    in_maps = []
    ttgs = []
    growss = []
    for c in range(NCORES):
        XT_dev, ttg, grows = _prep_core(X_train[c * NS : (c + 1) * NS])
        in_maps.append({"xT": xT_kio, "XT": XT_dev})
        ttgs.append(ttg)
        growss.append(grows)

    nc = _get_nc()
    kw = {}
    if _trace:
        kw = {"trace": True, "tmpdir": _tmpdir}
    res = run_bass_kernel_spmd(nc, in_maps, core_ids=list(range(NCORES)), **kw)

    cands = []
    for c in range(NCORES):
        sc = np.asarray(res.results[c]["scores"], dtype=np.float32)  # [B,4,1024]
        pooled = sc.reshape(B, NG)  # [B, NG], col G=v*1024+j
        est = pooled - 0.5 * ttgs[c][None, :]
        top = np.argpartition(-est, TOPK, axis=1)[:, :TOPK]  # [B, 8]
        rows = growss[c][top]  # [B, 8, 2] original shard rows
        cands.append(rows.reshape(B, TOPK * FOLD) + c * NS)
    cand = np.concatenate(cands, axis=1)  # [B, 128]
    out = _refine(xf, X_train, Y_train, cand)
    if _trace:
        return out, res
    return out


# revision 18
# speedup vs baseline: 1.1635x; 1.1635x over previous
"""Sharded top-1 KNN (retrieval) on 8 TRN2 NeuronCores via Bass/Tile.

v3 strategy (hardcoded for x[2048,24,16], X_train[65536,384], Y_train[65536,24,1]):
  - Shard X_train rows across 8 cores (8192 rows each), rows permuted so that
    each pooled output column covers tt-adjacent rows (tt = ||t||^2).
  - All scoring in fp8 e4m3 (ml_dtypes.float8_e4m3 == TRN FP8_EXP4): the
    384-dim contraction runs as one DoubleRow matmul (dims 0..255, 2 fp8
    packed per PE cell -> 2x throughput) plus one plain fp8 matmul
    (dims 256..383), accumulated in PSUM fp32.  Host-side recall check:
    fp8 quantization noise (std ~0.5) vs top-1->top-8 pooled margin (~11)
    leaves ~0 miss probability; exact distances are recomputed on host for
    the surviving candidates, so the final output is exact.
  - PSUM is drained with a Scalar/Vector split: ScalarE ACTIVATE-copies even
    fills psum->sbuf fp8, VectorE tensor_tensor-max folds odd fills onto
    them (the only engines that can read PSUM; GpSimd on TRN2 rejects
    tensor ops).  This emits fold-2 pooled score columns in fp8; top-8
    selection happens on HOST (saves MAX8/FIND_INDEX8 ~39us of VectorE
    time), after DMA-ing the pooled scores out (8.4MB/core).  Dummy
    matmuls during the input-DMA wait keep the PE's HAM activity monitor
    warm so real matmuls start at 2.4 GHz.
  - Host: rank pooled columns by P - mean_tt(group)/2, keep top-8 per core,
    expand 2 rows per column -> 8*8*2 = 128 candidates/query, recompute
    exact float64 distances, argmin (ties: smallest global index, matching
    jnp.argmin), return Y_train[best].
"""

import os
import sys

import numpy as np

for _p in ("/opt/trn_rl_repo",):
    if os.path.isdir(_p) and _p not in sys.path:
        sys.path.insert(0, _p)

import ml_dtypes  # noqa: E402

B, T, F = 2048, 24, 16
D = T * F  # 384
N = 65536
NCORES = 8
NS = N // NCORES  # 8192 rows per core
MT = B // 128  # 16 query tiles
NCHUNK = 512
NT = NS // NCHUNK  # 16 train chunks per core
NFILL = NT // 2  # 8 psum fills per m-tile, [128,1024] each
FOLD = 2  # rows pooled per output column (device-side)
NG = NS // FOLD  # 4096 pooled columns per core (per query)
TOPK = 8

_F8 = ml_dtypes.float8_e4m3
_BF16 = ml_dtypes.bfloat16


def build_nc(b=B, ns=NS):
    """Per-core Bass program (SPMD: same program, per-core inputs)."""
    import concourse.tile as tile
    from concourse import bacc, mybir

    mt = b // 128
    nt = ns // NCHUNK
    nfill = nt // 2

    nc = bacc.Bacc(None, target_bir_lowering=False)
    dt = mybir.dt
    # xT[ki, ko, b] = x[b, ko*128+ki]
    xT = nc.dram_tensor("xT", [128, 3, b], dt.float8e4, kind="ExternalInput")
    # XT[ki, ko, n] = Xs_dev[n, ko*128+ki]
    XT = nc.dram_tensor("XT", [128, 3, ns], dt.float8e4, kind="ExternalInput")
    # scores[q, v, j]: fill-pair v of query q's m-tile; fold-2 pooled column j
    OUT = nc.dram_tensor("scores", [b, nfill // 2, 1024], dt.float8e4,
                         kind="ExternalOutput")

    DR = mybir.MatmulPerfMode.DoubleRow
    MAX = mybir.AluOpType.max

    with tile.TileContext(nc) as tc:
        with (
            tc.tile_pool(name="wpool", bufs=1) as wpool,
            tc.tile_pool(name="ppool", bufs=1, space="PSUM") as ppool,
            tc.tile_pool(name="spool", bufs=3) as spool,
            tc.tile_pool(name="vpool", bufs=2) as vpool,
        ):
            # psum: 4 persistent tiles, rotated manually
            pts = [
                ppool.tile([128, 1024], dt.float32, name=f"pt{i}", tag=f"pt{i}")
                for i in range(4)
            ]
            # PE warmup: dummy matmuls during the input-DMA wait keep the HAM
            # activity monitor busy so real matmuls start at 2.4 GHz.
            dum = wpool.tile([128, 128], dt.float8e4, name="dum", tag="dum")
            nc.vector.memset(dum[:], 0.0)
            for i in range(40):
                nc.tensor.matmul(pts[0][:, 0:128], dum[:], dum[:],
                                 start=True, stop=True)

            xT_s = wpool.tile([128, 3, b], dt.float8e4, name="xT_s", tag="xT")
            nc.sync.dma_start(xT_s[:], xT[:])
            XT_s = wpool.tile([128, 3, ns], dt.float8e4, name="XT_s", tag="XT")
            # fill-granular input DMA so m=0 matmuls start early
            for f0 in range(nfill):
                csl = slice(f0 * 2 * NCHUNK, (f0 + 1) * 2 * NCHUNK)
                nc.sync.dma_start(XT_s[:, :, csl], XT[:, :, csl])

            for m in range(mt):
                msl = slice(m * 128, (m + 1) * 128)
                S = None
                V4 = vpool.tile([128, nfill // 2, 1024], dt.float8e4,
                                name="V4", tag="V4")
                for f in range(nfill):
                    pt = pts[f % 4]
                    for j in range(2):
                        c = 2 * f + j
                        csl = slice(c * NCHUNK, (c + 1) * NCHUNK)
                        dst = pt[:, j * NCHUNK : (j + 1) * NCHUNK]
                        nc.tensor.matmul(
                            dst, xT_s[:, 0:2, msl], XT_s[:, 0:2, csl],
                            start=True, stop=False, perf_mode=DR,
                        )
                        nc.tensor.matmul(
                            dst, xT_s[:, 2:3, msl], XT_s[:, 2:3, csl],
                            start=False, stop=True,
                        )
                    if f % 2 == 0:
                        S = spool.tile([128, 1024], dt.float8e4, name="S", tag="S")
                        nc.scalar.copy(S[:], pt[:])
                    else:
                        nc.vector.tensor_tensor(
                            V4[:, f // 2, :], pt[:], S[:], op=MAX
                        )
                if not os.environ.get("KNN_NO_OUT_DMA"):
                    nc.sync.dma_start(OUT[msl, :, :], V4[:])
    nc.finalize()
    return nc


_NC = None


def _get_nc():
    global _NC
    if _NC is None:
        _NC = build_nc()
    return _NC


def _group_rows(ng=NG):
    """Device rows covered by pooled column G (before the tt permutation).

    Pool col of output[v, q, j]: G = v*1024 + j; rows are chunks
    {4v+p, 4v+2+p} (p = j//512) at col j%512.
    """
    G = np.arange(ng)
    v, j = G // 1024, G % 1024
    p, col = j // 512, j % 512
    c0 = 4 * v + p
    c1 = 4 * v + 2 + p
    return np.stack([c0 * NCHUNK + col, c1 * NCHUNK + col], axis=1)  # [NG, 2]


def _rank_to_dev(ns=NS):
    """Device row for each tt-sorted rank r: group G=r//2 gets ranks 2G,2G+1."""
    r = np.arange(ns)
    G, i = r // FOLD, r % FOLD
    v, j = G // 1024, G % 1024
    p, col = j // 512, j % 512
    c = 4 * v + 2 * i + p
    return c * NCHUNK + col


_R2D = _rank_to_dev()
_GROWS = _group_rows()


def _prep_core(Xs):
    """Per-core device layout + host-side metadata."""
    Xq = Xs.astype(_F8)
    ttq = (Xq.astype(np.float64) ** 2).sum(axis=1)
    order = np.argsort(ttq, kind="stable")  # rank -> original shard row
    perm = np.empty(NS, dtype=np.int64)  # device row -> original shard row
    perm[_R2D] = order
    Xdev = Xq[perm]  # [NS, D] fp8
    XT_dev = np.ascontiguousarray(
        Xdev.T.reshape(3, 128, NS).transpose(1, 0, 2)
    )  # [128, 3, NS]
    tt_dev = ttq[perm]
    ttg = tt_dev[_GROWS].mean(axis=1)  # [NG] mean tt per pooled column
    grows = perm[_GROWS]  # [NG, 2] original shard rows per pooled column
    return XT_dev, ttg.astype(np.float32), grows


def _refine(xf, X_train, Y_train, cand):
    """cand: [B, C] global candidate row indices (may repeat)."""
    b = cand.shape[0]
    cand = np.sort(cand, axis=1)
    best = np.empty(b, dtype=np.int64)
    xd = xf.astype(np.float64)
    step = 256
    for s in range(0, b, step):
        e = min(s + step, b)
        Xc = X_train[cand[s:e]].astype(np.float64)  # [q, C, D]
        diff = xd[s:e, None, :] - Xc
        d2 = np.einsum("qcd,qcd->qc", diff, diff)
        best[s:e] = cand[s:e][np.arange(e - s), np.argmin(d2, axis=1)]
    return Y_train[best].astype(np.float32)


def kernel(x, X_train, Y_train, _trace=False, _tmpdir=None):
    from concourse.bass_utils import run_bass_kernel_spmd

    x = np.asarray(x, dtype=np.float32)
    X_train = np.asarray(X_train, dtype=np.float32)
    Y_train = np.asarray(Y_train, dtype=np.float32)
    xf = x.reshape(B, D)

    xq = xf.astype(_F8)
    xT_kio = np.ascontiguousarray(xq.T.reshape(3, 128, B).transpose(1, 0, 2))

    in_maps = []
    ttgs = []
    growss = []
    for c in range(NCORES):
        XT_dev, ttg, grows = _prep_core(X_train[c * NS : (c + 1) * NS])
        in_maps.append({"xT": xT_kio, "XT": XT_dev})
        ttgs.append(ttg)
        growss.append(grows)

    nc = _get_nc()
    kw = {}
    if _trace:
        kw = {"trace": True, "tmpdir": _tmpdir}
    res = run_bass_kernel_spmd(nc, in_maps, core_ids=list(range(NCORES)), **kw)

    cands = []
    for c in range(NCORES):
        sc = np.asarray(res.results[c]["scores"], dtype=np.float32)  # [B,4,1024]
        pooled = sc.reshape(B, NG)  # [B, NG], col G=v*1024+j
        est = pooled - 0.5 * ttgs[c][None, :]
        top = np.argpartition(-est, TOPK, axis=1)[:, :TOPK]  # [B, 8]
        rows = growss[c][top]  # [B, 8, 2] original shard rows
        cands.append(rows.reshape(B, TOPK * FOLD) + c * NS)
    cand = np.concatenate(cands, axis=1)  # [B, 128]
    out = _refine(xf, X_train, Y_train, cand)
    if _trace:
        return out, res
    return out
